# revision 1
# baseline (speedup 1.0000x reference)
"""Trainium2 Bass kernel for the per-game CriticNetwork (MoE-routed MLP).

Network (per sample b, with game g = idx[b]):
    h1  = relu(W1[g] @ state[b] + b1[g])          # [600]
    h2  = W2s @ h1 + b2s + W2a[g] @ action[b]     # [500]
    q   = W3[g] . relu(h2) + b3[g]                # scalar

Strategy: all MoE routing happens on the HOST. idx is (stably) sorted into
per-game contiguous segments, each segment is padded up to 512-sample tiles,
and the tile list is padded to a fixed 72 tiles (9 per core x 8 cores).
Every tile is single-game, so the device kernel is a fully static dense
pipeline; the host pre-gathers per-tile weight views (pre-transposed for the
PE's lhsT layout) so the device does zero routing and zero transposes.

Device per tile t (512 samples; matmul operands bf16 with fp32 PSUM
accumulation and fp32 biases). The PE charge per matmul is proportional to
the *moving free size* only, so the kernel avoids small-N / small-K matmuls:

  L1: 5 matmuls [K=128(d), M=128(h1 chunk), N=512(b)] + fused relu+bias.
      h1 is padded 600->640; the pad rows of the last chunk carry the
      action vector: relu writes partitions 0:112, a small DMA drops
      action.T into partitions 112:128 of the same slab.
  L2: 4 m-chunks x 5 accumulating matmuls. Chunks c=0..3 use the shared
      W2s.T; chunk c=4 uses a per-game combined block
      [W2s.T rows 512:600; zeros; W2a[g].T] folding the action term into
      the same chain (no separate K=16 W2a matmuls).
  L3: q = W3[g].relu(h2): 16 matmuls with lhsT = hf chunk [K=128, M=128
      samples] and rhs = one W3 column [K=128, N=1] -> out free size 1.
      They accumulate into a single persistent PSUM tile [128, 4*NT],
      drained by one copy + DMA at kernel end.

The emission order is a software-pipelined token schedule (CFG["template"]):
each tile's L2 c2/c3 groups, hf relus and L3 run during the NEXT tile's L1
phase, and relus are spread across ACT/DVE/Pool so neither PSUM-bank WAR
nor relu latency ever stalls the PE. b3 is added on the host after
gathering. Measured rel err vs the fp32 reference: ~2.9e-3.
"""

import numpy as np

import concourse.bass as bass
import concourse.mybir as mybir
import concourse.tile as tile
from concourse import bacc
from concourse.bass import ts
from concourse.bass_utils import run_bass_kernel_spmd

F32 = mybir.dt.float32
RELU = mybir.ActivationFunctionType.Relu

# Matmul operand dtype: bfloat16 runs the PE at 1 cycle/row (fp32 is 4).
MM_DT = mybir.dt.bfloat16
_NP_MM_DT = mybir.dt.np(MM_DT)

G = 8          # games
D = 128        # state dim
A = 16         # action dim
H1 = 600       # hidden 1 (padded to 640 = 5 * 128)
H2 = 500       # hidden 2 (padded to 512 = 4 * 128)
B = 32768      # batch
H1P, H2P = 640, 512
K1 = H1P // 128   # 5 h1 chunks
M2 = H2P // 128   # 4 h2 chunks
T = 512        # samples per tile (one PSUM bank of fp32)
NCORES = 8
NT = 9         # tiles per core; 72 total >= 64 + 7 worst-case segment padding
BPC = NT * T   # 4608 lanes per core
AROW = 112     # partition where action lands in the c4 h1 slab (112:128)
SROW = 88      # real W2s rows in the c4 chunk (h1 dims 512:600)

# Per-tile packed weight blob [128, WB] (bf16):
#   cols 0:640        w1t   (W1[g].T, d x h1p)
#   cols 640:644      w3t   (W3[g] chunk m at col 640+m)
#   cols 644:1156     w2sa  (combined L2 c4 chunk: rows 0:88 = W2s.T rows
#                            512:600, rows 112:128 = W2a[g].T)
W3OFF = H1P
C4OFF = H1P + M2
WB = H1P + M2 + H2P

# Token schedule per tile t (software-pipelined; "p" tokens refer to t-1):
#   ("l1", c)        L1 matmul chunk c
#   ("r1", c)        relu for L1 chunk c
#   ("l2", c, ms)    L2 chunk-c matmuls for m in ms (this tile)
#   ("l2p", c, ms)   same, previous tile
#   ("rhfp", m)      hf relu m, previous tile
#   ("l3p", s)       L3 slice chain s, previous tile
# Engines: 'A' = ACT, 'V' = DVE, 'P' = Pool/GPSIMD.
CFG = {
    "ps1_bufs": 3,
    "ps2_bufs": 4,
    "r1_eng": {4: 'A', 0: 'A', 1: 'V', 2: 'A', 3: 'V'},
    # NOTE: GPSIMD/Pool cannot read PSUM on HW (BIR verifier) — relus that
    # read PSUM banks must stay on ACT ('A') / DVE ('V').
    "rhf_eng": {0: 'A', 1: 'V', 2: 'A', 3: 'V'},
    "flush_rhf_eng": {0: 'A', 1: 'V', 2: 'A', 3: 'V'},
    "template": [
        ("l1", 4), ("r1", 4),
        ("l2p", 2, (0, 1, 2, 3)),
        ("l1", 0), ("r1", 0),
        ("l2p", 3, (0, 1, 2, 3)),
        ("rhfp", 0), ("rhfp", 1), ("rhfp", 2), ("rhfp", 3),
        ("l1", 1), ("r1", 1),
        ("l1", 2), ("r1", 2),
        ("l2", 4, (0, 1)),
        ("l1", 3), ("r1", 3),
        ("l2", 4, (2, 3)),
        ("l3p", 0), ("l3p", 1), ("l3p", 2), ("l3p", 3),
        ("l2", 0, (0, 1, 2, 3)),
        ("l2", 1, (0, 1, 2, 3)),
    ],
    "q_eng": 'V',        # final psq->sbuf copy engine
    # tile-0 state load goes via the Pool/SWDGE path so its transfer
    # overlaps the HWDGE-serialized weight loads at startup
    "t0_dma": ("wb1P", "st", "bc", "aT", "wb2", "w2stc01", "w2stc23"),
    "pe_warm": True,
    "early_drain": True,
    "t_dma": ("wb1", "st", "aT", "wb2"),
}

# Flush tokens for the last tile (no next tile to hide them in).
FLUSH = [
    ("l2p", 2, (0,)), ("l2p", 3, (0,)), ("rhfp", 0),
    ("l2p", 2, (1,)), ("l2p", 3, (1,)), ("rhfp", 1),
    ("l2p", 2, (2,)), ("l2p", 3, (2,)), ("rhfp", 2),
    ("l2p", 2, (3,)), ("l2p", 3, (3,)), ("rhfp", 3),
    ("l3p", 0), ("l3p", 1), ("l3p", 2), ("l3p", 3),
]

_NC = None


def build_nc():
    nc = bacc.Bacc("TRN2", target_bir_lowering=False, debug=False,
                   num_devices=NCORES)

    stateT = nc.declare_dram_parameter("stateT", [D, BPC], MM_DT, isOutput=False)
    aT = nc.declare_dram_parameter("aT", [A, BPC], MM_DT, isOutput=False)
    wblob = nc.declare_dram_parameter("wblob", [NT, 128, WB], MM_DT,
                                      isOutput=False)
    # biases packed in one fp32 block: cols 0:NT*K1 = b1 per tile, last M2
    # cols = b2s chunks
    bconst = nc.declare_dram_parameter("bconst", [128, NT * K1 + M2], F32,
                                       isOutput=False)
    w2st = nc.declare_dram_parameter("w2st", [512, H2P], MM_DT, isOutput=False)
    # q[j, 4t+s] = q of lane 512t + 128s + j
    q = nc.declare_dram_parameter("q", [128, M2 * NT], F32, isOutput=True)

    eng = {'A': lambda: nc.scalar, 'V': lambda: nc.vector,
           'P': lambda: nc.gpsimd}

    with tile.TileContext(nc) as tc:
        with (
            tc.tile_pool(name="const", bufs=1) as const,
            tc.tile_pool(name="wts", bufs=4) as wts,
            tc.tile_pool(name="acts", bufs=4) as acts,
            tc.tile_pool(name="hpool", bufs=3) as hpool,
            tc.tile_pool(name="outp", bufs=1) as outp,
            tc.tile_pool(name="ps1", bufs=CFG["ps1_bufs"], space="PSUM") as ps1p,
            tc.tile_pool(name="ps2", bufs=CFG["ps2_bufs"], space="PSUM") as ps2p,
            tc.tile_pool(name="psq", bufs=1, space="PSUM") as psqp,
        ):
            w2st_sb = const.tile([128, M2, H2P], MM_DT)
            bc_sb = const.tile([128, NT * K1 + M2], F32)
            psq = psqp.tile([128, M2 * NT], F32)
            # Warm-up: trigger the ACT-table load (~1.3us) during the
            # initial DMA dead time instead of before the first real relu.
            dumt = const.tile([1, 2], F32)
            nc.vector.memset(dumt[:], 0.0)
            nc.scalar.activation(dumt[:, 0:1], dumt[:, 1:2], RELU, bias=0.0)
            if CFG.get("pe_warm"):
                nc.tensor.matmul(psq[0:1, 0:1], dumt[0:1, 0:1],
                                 dumt[0:1, 1:2], start=True, stop=True)

            def relu_op(e, out, in_, bias):
                if e == 'A':
                    nc.scalar.activation(out, in_, RELU, bias=bias)
                else:
                    eng[e]().tensor_scalar(out, in_, bias, 0.0,
                                           mybir.AluOpType.add,
                                           mybir.AluOpType.max)

            class Tile:
                def __init__(self, t):
                    self.t = t
                    self.wb = wts.tile([128, WB], MM_DT, tag="wb")
                    self.st = acts.tile([D, T], MM_DT, tag="st")
                    self.h1 = hpool.tile([128, K1, T], MM_DT, tag="h1")
                    dmas = {
                        "wb1": lambda: nc.sync.dma_start(
                            self.wb[:, 0:C4OFF], wblob[t][:, 0:C4OFF]),
                        "wb2": lambda: nc.sync.dma_start(
                            self.wb[:, C4OFF:], wblob[t][:, C4OFF:]),
                        "wb": lambda: nc.sync.dma_start(self.wb[:], wblob[t]),
                        "st": lambda: nc.sync.dma_start(
                            self.st[:], stateT[:, ts(t, T)]),
                        "stP": lambda: nc.gpsimd.dma_start(
                            self.st[:], stateT[:, ts(t, T)]),
                        "wb1P": lambda: nc.gpsimd.dma_start(
                            self.wb[:, 0:C4OFF], wblob[t][:, 0:C4OFF]),
                        "w2stc01": lambda: nc.sync.dma_start(
                            w2st_sb[:, 0:2, :], w2st.ap()[0:256, :].rearrange(
                                "(c p) n -> p c n", p=128)),
                        "w2stc23": lambda: nc.sync.dma_start(
                            w2st_sb[:, 2:4, :], w2st.ap()[256:512, :].rearrange(
                                "(c p) n -> p c n", p=128)),
                        "aT": lambda: nc.sync.dma_start(
                            self.h1[AROW:AROW + A, K1 - 1, :],
                            aT[:, ts(t, T)]),
                        "bc": lambda: nc.sync.dma_start(bc_sb[:], bconst.ap()),
                        "w2st": lambda: nc.sync.dma_start(
                            w2st_sb[:],
                            w2st.ap().rearrange("(c p) n -> p c n", p=128)),
                        **{f"w2stc{c}": (lambda c=c: nc.sync.dma_start(
                            w2st_sb[:, c, :],
                            w2st.ap()[128 * c:128 * (c + 1), :]))
                           for c in range(M2)},
                    }
                    order = CFG["t0_dma"] if t == 0 else CFG["t_dma"]
                    for d in order:
                        dmas[d]()
                    self.hf = hpool.tile([128, M2, T], MM_DT, tag="hf")
                    self.ps1 = {}
                    self.ps2 = {}
                    self.n_l2 = {m: 0 for m in range(M2)}

                def l1(self, c):
                    ps1c = ps1p.tile([128, T], F32, tag="ps1")
                    p = self.ps1[c] = ps1c
                    nc.tensor.matmul(p[:], self.wb[:, ts(c, 128)], self.st[:],
                                     start=True, stop=True)

                def r1(self, c, e):
                    p = self.ps1.pop(c)
                    b = bc_sb[:, self.t * K1 + c:self.t * K1 + c + 1]
                    if c == K1 - 1:
                        relu_op(e, self.h1[0:AROW, c, :], p[0:AROW, :],
                                b[0:AROW, :])
                    else:
                        relu_op(e, self.h1[:, c, :], p[:], b)

                def l2(self, c, ms):
                    for m in ms:
                        if m not in self.ps2:
                            ps2m = ps2p.tile([128, T], F32, tag="ps2")
                            self.ps2[m] = ps2m
                        n = self.n_l2[m]
                        self.n_l2[m] = n + 1
                        if c == K1 - 1:
                            lhsT = self.wb[:, C4OFF + 128 * m:
                                           C4OFF + 128 * (m + 1)]
                        else:
                            lhsT = w2st_sb[:, c, ts(m, 128)]
                        nc.tensor.matmul(self.ps2[m][:], lhsT,
                                         self.h1[:, c, :],
                                         start=(n == 0), stop=(n == K1 - 1))

                def rhf(self, m, e):
                    p = self.ps2.pop(m)
                    b = bc_sb[:, NT * K1 + m:NT * K1 + m + 1]
                    if isinstance(e, tuple):
                        h = T // 2
                        relu_op(e[0], self.hf[:, m, 0:h], p[:, 0:h], b)
                        relu_op(e[1], self.hf[:, m, h:T], p[:, h:T], b)
                    else:
                        relu_op(e, self.hf[:, m, :], p[:], b)

                def l3(self, s):
                    col = M2 * self.t + s
                    for m in range(M2):
                        nc.tensor.matmul(psq[:, col:col + 1],
                                         self.hf[:, m, ts(s, 128)],
                                         self.wb[:, W3OFF + m:W3OFF + m + 1],
                                         start=(m == 0), stop=(m == M2 - 1))

            def run_token(tok, cur, prev, flush=False):
                kind = tok[0]
                if kind == "l1":
                    cur.l1(tok[1])
                elif kind == "r1":
                    cur.r1(tok[1], CFG["r1_eng"][tok[1]])
                elif kind == "l2":
                    cur.l2(tok[1], tok[2])
                elif kind == "l2p":
                    if prev is not None:
                        prev.l2(tok[1], tok[2])
                elif kind == "rhf":
                    cur.rhf(tok[1], CFG["rhf_eng"][tok[1]])
                elif kind == "l3":
                    cur.l3(tok[1])
                elif kind == "rhfp":
                    if prev is not None:
                        emap = CFG["flush_rhf_eng"] if flush else CFG["rhf_eng"]
                        prev.rhf(tok[1], emap[tok[1]])
                elif kind == "l3p":
                    if prev is not None:
                        prev.l3(tok[1])
                else:
                    raise ValueError(tok)

            prev = None
            for t in range(NT):
                cur = Tile(t)
                tmpl = CFG["template"]
                if t == NT - 1 and CFG.get("template_last"):
                    tmpl = CFG["template_last"]
                for tok in tmpl:
                    run_token(tok, cur, prev)
                prev = cur
            q_sb = outp.tile([128, M2 * NT], F32, tag="q")
            ned = M2 * (NT - 1)   # columns final before the last tile drains
            if CFG.get("early_drain"):
                # tiles 0..NT-2 are fully accumulated once the last tile's
                # l3p tokens ran — drain them while the flush computes
                eng[CFG["q_eng"]]().tensor_copy(q_sb[:, 0:ned],
                                                psq[:, 0:ned])
                nc.sync.dma_start(q.ap()[:, 0:ned], q_sb[:, 0:ned])
            if not CFG.get("template_last"):
                for tok in FLUSH:
                    run_token(tok, None, prev, flush=True)
            e = CFG["q_eng"]
            lo = ned if CFG.get("early_drain") else 0
            if e == 'A':
                nc.scalar.activation(q_sb[:, lo:], psq[:, lo:],
                                     mybir.ActivationFunctionType.Copy)
            else:
                eng[e]().tensor_copy(q_sb[:, lo:], psq[:, lo:])
            nc.sync.dma_start(q.ap()[:, lo:], q_sb[:, lo:])

    nc.compile()
    return nc


def _get_nc():
    global _NC
    if _NC is None:
        _NC = build_nc()
    return _NC


def _plan_tiles(idx):
    """Stable-sort samples by game, pad each game segment to 512-sample
    tiles, pad the tile list to the fixed 72. Returns (sel, valid, gids):
    sel[t, l] = original sample index feeding lane l of tile t."""
    perm = np.argsort(idx, kind="stable")
    counts = np.bincount(idx, minlength=G)
    ntot = NCORES * NT
    sel = np.zeros((ntot, T), np.int64)
    valid = np.zeros((ntot, T), bool)
    gids = np.zeros(ntot, np.int64)
    pos, t = 0, 0
    for g in range(G):
        cg = int(counts[g])
        for k in range((cg + T - 1) // T):
            n = min(T, cg - k * T)
            lanes = perm[pos:pos + n]
            sel[t, :n] = lanes
            valid[t, :n] = True
            if n < T:
                sel[t, n:] = lanes[0]
            gids[t] = g
            pos += n
            t += 1
    assert t <= ntot, f"tile plan overflow: {t} > {ntot}"
    return sel, valid, gids


def build_in_maps(inputs):
    state = np.ascontiguousarray(np.asarray(inputs["state"], np.float32))
    action = np.ascontiguousarray(np.asarray(inputs["action"], np.float32))
    idx = np.asarray(inputs["idx"]).astype(np.int64)
    W1 = np.asarray(inputs["W1"], np.float32)
    b1 = np.asarray(inputs["b1"], np.float32)
    W2s = np.asarray(inputs["W2s"], np.float32)
    b2s = np.asarray(inputs["b2s"], np.float32)
    W2a = np.asarray(inputs["W2a"], np.float32)
    W3 = np.asarray(inputs["W3"], np.float32)
    assert state.shape == (B, D) and action.shape == (B, A)

    sel, valid, gids = _plan_tiles(idx)

    # Pre-transposed / padded weight views, indexed per tile by game id.
    W1T_all = np.zeros((G, D, H1P), np.float32)
    W1T_all[:, :, :H1] = W1.transpose(0, 2, 1)
    b1P = np.zeros((G, H1P), np.float32)
    b1P[:, :H1] = b1
    b1c_all = np.ascontiguousarray(b1P.reshape(G, K1, 128).transpose(0, 2, 1))
    W2sTP = np.zeros((H1P, H2P), np.float32)
    W2sTP[:H1, :H2] = W2s.T
    W2aT_all = np.zeros((G, A, H2P), np.float32)
    W2aT_all[:, :, :H2] = W2a.transpose(0, 2, 1)
    b2sP = np.zeros(H2P, np.float32)
    b2sP[:H2] = b2s
    b2st = np.ascontiguousarray(b2sP.reshape(M2, 128).T)
    W3P = np.zeros((G, H2P), np.float32)
    W3P[:, :H2] = W3
    W3T_all = np.ascontiguousarray(W3P.reshape(G, M2, 128).transpose(0, 2, 1))
    # Combined per-game L2 c4 chunk: W2s tail rows + action rows.
    w2sa_all = np.zeros((G, 128, H2P), np.float32)
    w2sa_all[:, 0:SROW, :] = W2sTP[512:512 + SROW, :]
    w2sa_all[:, AROW:AROW + A, :] = W2aT_all

    # Per-game packed weight blob (layout documented at top).
    blob_all = np.zeros((G, 128, WB), np.float32)
    blob_all[:, :, 0:H1P] = W1T_all
    blob_all[:, :, W3OFF:W3OFF + M2] = W3T_all
    blob_all[:, :, C4OFF:] = w2sa_all
    blob_all = blob_all.astype(_NP_MM_DT)

    in_maps = []
    for c in range(NCORES):
        tsl = slice(c * NT, (c + 1) * NT)
        lanes = sel[tsl].reshape(-1)
        gt = gids[tsl]
        # biases for all 9 tiles + b2s chunks as one [128, NT*K1+M2] block
        bconst = np.concatenate(
            [b1c_all[gt].transpose(1, 0, 2).reshape(128, NT * K1), b2st],
            axis=1)
        in_maps.append({
            "stateT": np.ascontiguousarray(state[lanes].T).astype(_NP_MM_DT),
            "aT": np.ascontiguousarray(action[lanes].T).astype(_NP_MM_DT),
            "wblob": np.ascontiguousarray(blob_all[gt]),
            "bconst": np.ascontiguousarray(bconst),
            "w2st": W2sTP[:512].astype(_NP_MM_DT),
        })
    return in_maps, sel, valid


def kernel(**inputs):
    idx = np.asarray(inputs["idx"]).astype(np.int64)
    b3 = np.asarray(inputs["b3"], np.float32)
    in_maps, sel, valid = build_in_maps(inputs)

    res = run_bass_kernel_spmd(_get_nc(), in_maps, list(range(NCORES))).results
    # q[j, 4t+s] = lane 512t + 128s + j  ->  [t, s, j] order
    qv = np.concatenate([
        np.asarray(res[c]["q"]).reshape(128, NT, M2).transpose(1, 2, 0)
        .reshape(-1)
        for c in range(NCORES)])

    out = np.zeros(B, np.float32)
    flat_sel = sel.reshape(-1)
    flat_valid = valid.reshape(-1)
    out[flat_sel[flat_valid]] = qv[flat_valid]
    out += b3[idx]
    return out.astype(np.float32)



# revision 4
# speedup vs baseline: 1.5210x; 1.5210x over previous
"""Trainium2 Bass kernel for the per-game CriticNetwork (MoE-routed MLP).

Network (per sample b, with game g = idx[b]):
    h1  = relu(W1[g] @ state[b] + b1[g])          # [600]
    h2  = W2s @ h1 + b2s + W2a[g] @ action[b]     # [500]
    q   = W3[g] . relu(h2) + b3[g]                # scalar

Strategy: all MoE routing happens on the HOST. idx is (stably) sorted into
per-game contiguous segments, each segment is padded up to 512-sample tiles,
and the tile list is padded to a fixed 72 tiles (9 per core x 8 cores).
Every tile is single-game, so the device kernel is a fully static dense
pipeline; the host pre-gathers per-tile weight views (pre-transposed for the
PE's lhsT layout) so the device does zero routing and zero transposes.

Precision/speed layout (per 512-sample tile):
  L1 (state->h1 pre-act) runs in bf16: 5 matmuls [K=128, M=128, N=512].
      W1 is host-scaled by SH=32 (exact power of 2 in bf16) so the relu
      output h1*32 lands in fp8e4m3's normal range.
  r1: relu+bias (fp32 PSUM -> fp8 SBUF) writes the h1 slab [128, 6, 512]:
      k-tiles 0..4 = 32*h1, k-tile 5 = host-DMA'd action rows.
  L2 (h1->h2) runs in fp8 DoubleRow perf mode (0.5 PE cycles/row, 2 k-tiles
      of 128 per instruction): per m-chunk 3 DR matmuls with
      lhsT = fp8(64*W2s^T) k-tile pairs (0,1), (2,3), (4, action-block).
      The action term is folded in with first-order error compensation:
      k-tile5 rows = [Whi;Whi;Wlo] vs rhs rows [ahi;alo;ahi] where
      Whi/Wlo = fp8 hi/lo split of 64*W2a[g]^T and ahi/alo of 32*action.
      PSUM accumulates 2048*h2.
  rhf: relu (+2048*b2s bias) -> hf in bf16 (scaled by 2048; exact).
  L3: q*2048 = W3^T(bf16) . hf: 16 matmuls with lhsT = hf chunk [K=128,
      M=128 samples], rhs = one W3 column [K=128, N=1] -> out free size 1,
      accumulated into one persistent PSUM tile [128, 4*NT], drained at end.
  Host divides by 2048 and adds b3.

The emission order is a software-pipelined token schedule (CFG["template"]):
tile t's L2 pairs p1/p2, hf relus and L3 run during tile t+1's L1 phase, and
relus are spread across ACT/DVE so neither PSUM-bank WAR nor relu latency
stalls the PE. Measured rel err vs the fp32 reference: ~1.1e-2 (numpy fp8
emulation; gate is 2e-2).
"""

import numpy as np

import concourse.bass as bass
import concourse.mybir as mybir
import concourse.tile as tile
from concourse import bacc
from concourse.bass import ts
from concourse.bass_utils import run_bass_kernel_spmd

F32 = mybir.dt.float32
BF16 = mybir.dt.bfloat16
FP8 = mybir.dt.float8e4
RELU = mybir.ActivationFunctionType.Relu
DR = mybir.MatmulPerfMode.DoubleRow

_NP_BF16 = mybir.dt.np(BF16)
_NP_FP8 = mybir.dt.np(FP8)

G = 8          # games
D = 128        # state dim
A = 16         # action dim
H1 = 600       # hidden 1 (padded to 640 = 5 * 128)
H2 = 500       # hidden 2 (padded to 512 = 4 * 128)
B = 32768      # batch
H1P, H2P = 640, 512
K1 = H1P // 128   # 5 h1 chunks
K1A = K1 + 1      # h1 slab k-tiles incl action k-tile
M2 = H2P // 128   # 4 h2 chunks
NP2 = 3           # DR k-tile pairs per m-chunk
T = 512        # samples per tile (one PSUM bank of fp32)
NCORES = 8
NT = 9         # tiles per core; 72 total >= 64 + 7 worst-case segment padding
BPC = NT * T   # 4608 lanes per core

SH = 32.0      # h1 / action scale (power of 2)
SW = 64.0      # W2s / W2a scale (power of 2)
SP2 = SH * SW  # h2 PSUM scale

# wb16 blob [128, WB16] (bf16): cols 0:640 w1t (32*W1[g].T), cols 640:644 w3t
W3OFF = H1P
WB16 = H1P + M2

# Token schedule per tile t (software-pipelined; "p" tokens refer to t-1):
#   ("l1", c)        L1 matmul chunk c
#   ("r1", c)        relu for L1 chunk c -> h1 slab k-tile c (fp8)
#   ("l2", p, ms)    L2 DR pair-p matmuls for m in ms (this tile)
#   ("l2p", p, ms)   same, previous tile
#   ("rhfp", m)      hf relu m, previous tile
#   ("l3p", s)       L3 slice chain s, previous tile
# Engines: 'A' = ACT, 'V' = DVE.
CFG = {
    "ps1_bufs": 3,
    "ps2_bufs": 4,
    "r1_eng": {0: 'A', 1: 'V', 2: 'A', 3: 'V', 4: 'A'},
    "rhf_eng": {0: 'V', 1: 'A', 2: 'V', 3: ('A', 'V')},
    "flush_rhf_eng": {0: 'A', 1: 'V', 2: 'A', 3: 'V'},
    "template": [
        ("l1", 0), ("r1", 0),
        ("l2p", 1, (0, 1, 2, 3)),
        ("l1", 1), ("r1", 1),
        ("l2p", 2, (0, 1, 2, 3)),
        ("l1", 2), ("r1", 2),
        ("rhfp", 0), ("rhfp", 1),
        ("l1", 3), ("r1", 3),
        ("rhfp", 2), ("rhfp", 3),
        ("l1", 4), ("r1", 4),
        ("l2", 0, (0, 1, 2, 3)),
        ("l3p", 0), ("l3p", 1), ("l3p", 2), ("l3p", 3),
    ],
    "q_eng": 'V',        # final psq->sbuf copy engine
    "t0_dma": ("st", "wb16", "wb8P", "aT8P", "w2s8", "bc"),
    "pe_warm": True,
    "early_drain": True,
    "t_dma": ("st", "wb16", "wb8", "aT8P"),
}

# Flush tokens for the last tile (no next tile to hide them in).
FLUSH = [
    ("l2p", 1, (0, 1, 2, 3)),
    ("l2p", 2, (0, 1, 2, 3)),
    ("rhfp", 0), ("rhfp", 1), ("rhfp", 2), ("rhfp", 3),
    ("l3p", 0), ("l3p", 1), ("l3p", 2), ("l3p", 3),
]

_NC = None


def build_nc():
    nc = bacc.Bacc("TRN2", target_bir_lowering=False, debug=False,
                   num_devices=NCORES)

    stateT = nc.declare_dram_parameter("stateT", [D, BPC], BF16, isOutput=False)
    wb16 = nc.declare_dram_parameter("wb16", [NT, 128, WB16], BF16,
                                     isOutput=False)
    # per-tile L2 pair-2 lhsT: [m, i(2), col(128)]
    wb8 = nc.declare_dram_parameter("wb8", [NT, 128, M2, 2, 128], FP8,
                                    isOutput=False)
    # per-tile action rows for the h1 slab k-tile 5: [ahi;alo;ahi;0...]
    aT8 = nc.declare_dram_parameter("aT8", [NT, 128, T], FP8, isOutput=False)
    # shared L2 pairs 0,1 lhsT: [m, p(2), i(2), col(128)]
    w2s8 = nc.declare_dram_parameter("w2s8", [128, M2, 2, 2, 128], FP8,
                                     isOutput=False)
    # biases packed in one fp32 block: cols 0:NT*K1 = 32*b1 per tile, last M2
    # cols = 2048*b2s chunks
    bconst = nc.declare_dram_parameter("bconst", [128, NT * K1 + M2], F32,
                                       isOutput=False)
    # q[j, 4t+s] = 2048*q of lane 512t + 128s + j
    q = nc.declare_dram_parameter("q", [128, M2 * NT], F32, isOutput=True)

    eng = {'A': lambda: nc.scalar, 'V': lambda: nc.vector,
           'P': lambda: nc.gpsimd}

    with tile.TileContext(nc) as tc:
        with (
            tc.tile_pool(name="const", bufs=1) as const,
            tc.tile_pool(name="wts", bufs=4) as wts,
            tc.tile_pool(name="acts", bufs=4) as acts,
            tc.tile_pool(name="hpool", bufs=3) as hpool,
            tc.tile_pool(name="outp", bufs=1) as outp,
            tc.tile_pool(name="ps1", bufs=CFG["ps1_bufs"], space="PSUM") as ps1p,
            tc.tile_pool(name="ps2", bufs=CFG["ps2_bufs"], space="PSUM") as ps2p,
            tc.tile_pool(name="psq", bufs=1, space="PSUM") as psqp,
        ):
            w2s8_sb = const.tile([128, M2, 2, 2, 128], FP8)
            bc_sb = const.tile([128, NT * K1 + M2], F32)
            psq = psqp.tile([128, M2 * NT], F32)
            # Warm-up: trigger the ACT-table load (~1.3us) during the
            # initial DMA dead time instead of before the first real relu.
            dumt = const.tile([1, 2], F32)
            nc.vector.memset(dumt[:], 0.0)
            nc.scalar.activation(dumt[:, 0:1], dumt[:, 1:2], RELU, bias=0.0)
            if CFG.get("pe_warm"):
                nc.tensor.matmul(psq[0:1, 0:1], dumt[0:1, 0:1],
                                 dumt[0:1, 1:2], start=True, stop=True)

            def relu_op(e, out, in_, bias):
                if e == 'A':
                    nc.scalar.activation(out, in_, RELU, bias=bias)
                else:
                    eng[e]().tensor_scalar(out, in_, bias, 0.0,
                                           mybir.AluOpType.add,
                                           mybir.AluOpType.max)

            class Tile:
                def __init__(self, t):
                    self.t = t
                    self.wb = wts.tile([128, WB16], BF16, tag="wb")
                    self.w8 = wts.tile([128, M2, 2, 128], FP8, tag="w8")
                    self.st = acts.tile([D, T], BF16, tag="st")
                    # h1 slab: k-tiles 0..4 = 32*h1 (fp8), k-tile 5 = action
                    self.h1 = hpool.tile([128, K1A, T], FP8, tag="h1")
                    dmas = {
                        "wb16": lambda: nc.sync.dma_start(self.wb[:], wb16[t]),
                        "wb8": lambda: nc.sync.dma_start(self.w8[:], wb8[t]),
                        "wb8P": lambda: nc.gpsimd.dma_start(self.w8[:], wb8[t]),
                        "st": lambda: nc.sync.dma_start(
                            self.st[:], stateT[:, ts(t, T)]),
                        "aT8": lambda: nc.sync.dma_start(
                            self.h1[:, K1, :], aT8[t]),
                        "aT8P": lambda: nc.gpsimd.dma_start(
                            self.h1[:, K1, :], aT8[t]),
                        "w2s8": lambda: nc.sync.dma_start(
                            w2s8_sb[:], w2s8.ap()),
                        "bc": lambda: nc.sync.dma_start(bc_sb[:], bconst.ap()),
                    }
                    order = CFG["t0_dma"] if t == 0 else CFG["t_dma"]
                    for d in order:
                        dmas[d]()
                    self.hf = hpool.tile([128, M2, T], BF16, tag="hf")
                    self.ps1 = {}
                    self.ps2 = {}

                def l1(self, c):
                    ps1c = ps1p.tile([128, T], F32, tag="ps1")
                    p = self.ps1[c] = ps1c
                    nc.tensor.matmul(p[:], self.wb[:, ts(c, 128)], self.st[:],
                                     start=True, stop=True)

                def r1(self, c, e):
                    p = self.ps1.pop(c)
                    b = bc_sb[:, self.t * K1 + c:self.t * K1 + c + 1]
                    relu_op(e, self.h1[:, c, :], p[:], b)

                def l2(self, p, ms):
                    for m in ms:
                        if p == 0:
                            ps2m = ps2p.tile([128, T], F32, tag="ps2")
                            self.ps2[m] = ps2m
                        if p == NP2 - 1:
                            lhsT = self.w8[:, m, :, :]
                        else:
                            lhsT = w2s8_sb[:, m, p, :, :]
                        nc.tensor.matmul(self.ps2[m][:], lhsT,
                                         self.h1[:, 2 * p:2 * p + 2, :],
                                         start=(p == 0), stop=(p == NP2 - 1),
                                         perf_mode=DR)

                def rhf(self, m, e):
                    p = self.ps2.pop(m)
                    b = bc_sb[:, NT * K1 + m:NT * K1 + m + 1]
                    if isinstance(e, tuple):
                        h = T // 2
                        relu_op(e[0], self.hf[:, m, 0:h], p[:, 0:h], b)
                        relu_op(e[1], self.hf[:, m, h:T], p[:, h:T], b)
                    else:
                        relu_op(e, self.hf[:, m, :], p[:], b)

                def l3(self, s):
                    col = M2 * self.t + s
                    for m in range(M2):
                        nc.tensor.matmul(psq[:, col:col + 1],
                                         self.hf[:, m, ts(s, 128)],
                                         self.wb[:, W3OFF + m:W3OFF + m + 1],
                                         start=(m == 0), stop=(m == M2 - 1))

            def run_token(tok, cur, prev, flush=False):
                kind = tok[0]
                if kind == "l1":
                    cur.l1(tok[1])
                elif kind == "r1":
                    cur.r1(tok[1], CFG["r1_eng"][tok[1]])
                elif kind == "l2":
                    cur.l2(tok[1], tok[2])
                elif kind == "l2p":
                    if prev is not None:
                        prev.l2(tok[1], tok[2])
                elif kind == "rhf":
                    cur.rhf(tok[1], CFG["rhf_eng"][tok[1]])
                elif kind == "l3":
                    cur.l3(tok[1])
                elif kind == "rhfp":
                    if prev is not None:
                        emap = CFG["flush_rhf_eng"] if flush else CFG["rhf_eng"]
                        prev.rhf(tok[1], emap[tok[1]])
                elif kind == "l3p":
                    if prev is not None:
                        prev.l3(tok[1])
                else:
                    raise ValueError(tok)

            prev = None
            for t in range(NT):
                cur = Tile(t)
                tmpl = CFG["template"]
                if t == NT - 1 and CFG.get("template_last"):
                    tmpl = CFG["template_last"]
                for tok in tmpl:
                    run_token(tok, cur, prev)
                prev = cur
            q_sb = outp.tile([128, M2 * NT], F32, tag="q")
            ned = M2 * (NT - 1)   # columns final before the last tile drains
            if CFG.get("early_drain"):
                # tiles 0..NT-2 are fully accumulated once the last tile's
                # l3p tokens ran — drain them while the flush computes
                eng[CFG["q_eng"]]().tensor_copy(q_sb[:, 0:ned],
                                                psq[:, 0:ned])
                nc.sync.dma_start(q.ap()[:, 0:ned], q_sb[:, 0:ned])
            if not CFG.get("template_last"):
                for tok in FLUSH:
                    run_token(tok, None, prev, flush=True)
            e = CFG["q_eng"]
            lo = ned if CFG.get("early_drain") else 0
            if e == 'A':
                nc.scalar.activation(q_sb[:, lo:], psq[:, lo:],
                                     mybir.ActivationFunctionType.Copy)
            else:
                eng[e]().tensor_copy(q_sb[:, lo:], psq[:, lo:])
            nc.sync.dma_start(q.ap()[:, lo:], q_sb[:, lo:])

    nc.compile()
    return nc


def _get_nc():
    global _NC
    if _NC is None:
        _NC = build_nc()
    return _NC


def _plan_tiles(idx):
    """Stable-sort samples by game, pad each game segment to 512-sample
    tiles, pad the tile list to the fixed 72. Returns (sel, valid, gids):
    sel[t, l] = original sample index feeding lane l of tile t."""
    perm = np.argsort(idx, kind="stable")
    counts = np.bincount(idx, minlength=G)
    ntot = NCORES * NT
    sel = np.zeros((ntot, T), np.int64)
    valid = np.zeros((ntot, T), bool)
    gids = np.zeros(ntot, np.int64)
    pos, t = 0, 0
    for g in range(G):
        cg = int(counts[g])
        for k in range((cg + T - 1) // T):
            n = min(T, cg - k * T)
            lanes = perm[pos:pos + n]
            sel[t, :n] = lanes
            valid[t, :n] = True
            if n < T:
                sel[t, n:] = lanes[0]
            gids[t] = g
            pos += n
            t += 1
    assert t <= ntot, f"tile plan overflow: {t} > {ntot}"
    return sel, valid, gids


def _f8(x):
    return np.asarray(x, np.float32).astype(_NP_FP8)


def build_in_maps(inputs):
    state = np.ascontiguousarray(np.asarray(inputs["state"], np.float32))
    action = np.ascontiguousarray(np.asarray(inputs["action"], np.float32))
    idx = np.asarray(inputs["idx"]).astype(np.int64)
    W1 = np.asarray(inputs["W1"], np.float32)
    b1 = np.asarray(inputs["b1"], np.float32)
    W2s = np.asarray(inputs["W2s"], np.float32)
    b2s = np.asarray(inputs["b2s"], np.float32)
    W2a = np.asarray(inputs["W2a"], np.float32)
    W3 = np.asarray(inputs["W3"], np.float32)
    assert state.shape == (B, D) and action.shape == (B, A)

    sel, valid, gids = _plan_tiles(idx)

    # L1 weights: 32*W1^T in bf16, padded 600->640.
    W1T_all = np.zeros((G, D, H1P), np.float32)
    W1T_all[:, :, :H1] = W1.transpose(0, 2, 1) * SH
    b1P = np.zeros((G, H1P), np.float32)
    b1P[:, :H1] = b1 * SH
    b1c_all = np.ascontiguousarray(b1P.reshape(G, K1, 128).transpose(0, 2, 1))

    # L2 shared weights: fp8(64*W2s^T), k-tiles arranged per (m, pair, i).
    W2sTP = np.zeros((H1P, H2P), np.float32)
    W2sTP[:H1, :H2] = W2s.T * SW
    W2sq = _f8(W2sTP)  # [640, 512]
    # [kt, 128, m, col] -> [128, m, p, i, col]
    w2skt = W2sq.reshape(K1, 128, M2, 128)
    w2s8_host = np.zeros((128, M2, 2, 2, 128), _NP_FP8)
    for p in range(2):
        for i in range(2):
            w2s8_host[:, :, p, i, :] = w2skt[2 * p + i]

    # Per-game pair-2 lhsT: i=0 -> W2s k-tile 4; i=1 -> action block
    # rows [Whi(16); Whi(16); Wlo(16); 0].
    W2aT_all = np.zeros((G, A, H2P), np.float32)
    W2aT_all[:, :, :H2] = W2a.transpose(0, 2, 1) * SW
    Wahi = _f8(W2aT_all)
    Walo = _f8(W2aT_all - Wahi.astype(np.float32))
    wb8_all = np.zeros((G, 128, M2, 2, 128), _NP_FP8)
    for m in range(M2):
        wb8_all[:, :, m, 0, :] = w2skt[4][None, :, m, :]
        wb8_all[:, 0:A, m, 1, :] = Wahi[:, :, 128 * m:128 * (m + 1)]
        wb8_all[:, A:2 * A, m, 1, :] = Wahi[:, :, 128 * m:128 * (m + 1)]
        wb8_all[:, 2 * A:3 * A, m, 1, :] = Walo[:, :, 128 * m:128 * (m + 1)]

    b2sP = np.zeros(H2P, np.float32)
    b2sP[:H2] = b2s * SP2
    b2st = np.ascontiguousarray(b2sP.reshape(M2, 128).T)

    W3P = np.zeros((G, H2P), np.float32)
    W3P[:, :H2] = W3
    W3T_all = np.ascontiguousarray(W3P.reshape(G, M2, 128).transpose(0, 2, 1))

    # wb16 per-game blob: [128, 640 w1t + 4 w3t] bf16
    wb16_all = np.zeros((G, 128, WB16), np.float32)
    wb16_all[:, :, 0:H1P] = W1T_all
    wb16_all[:, :, W3OFF:W3OFF + M2] = W3T_all
    wb16_all = wb16_all.astype(_NP_BF16)

    # Action rows (per sample): [ahi; alo; ahi; 0...] of 32*action^T.
    aS = action * SH
    ahi_all = _f8(aS)                                    # [B, 16]
    alo_all = _f8(aS - ahi_all.astype(np.float32))       # [B, 16]

    in_maps = []
    for c in range(NCORES):
        tsl = slice(c * NT, (c + 1) * NT)
        lanes = sel[tsl].reshape(-1)
        gt = gids[tsl]
        bconst = np.concatenate(
            [b1c_all[gt].transpose(1, 0, 2).reshape(128, NT * K1), b2st],
            axis=1)
        aT8_host = np.zeros((NT, 128, T), _NP_FP8)
        ahi_c = ahi_all[lanes].reshape(NT, T, A)
        alo_c = alo_all[lanes].reshape(NT, T, A)
        for t in range(NT):
            aT8_host[t, 0:A] = ahi_c[t].T
            aT8_host[t, A:2 * A] = alo_c[t].T
            aT8_host[t, 2 * A:3 * A] = ahi_c[t].T
        in_maps.append({
            "stateT": np.ascontiguousarray(state[lanes].T).astype(_NP_BF16),
            "wb16": np.ascontiguousarray(wb16_all[gt]),
            "wb8": np.ascontiguousarray(wb8_all[gt]),
            "aT8": aT8_host,
            "w2s8": w2s8_host,
            "bconst": np.ascontiguousarray(bconst),
        })
    return in_maps, sel, valid


def kernel(**inputs):
    idx = np.asarray(inputs["idx"]).astype(np.int64)
    b3 = np.asarray(inputs["b3"], np.float32)
    in_maps, sel, valid = build_in_maps(inputs)

    res = run_bass_kernel_spmd(_get_nc(), in_maps, list(range(NCORES))).results
    # q[j, 4t+s] = lane 512t + 128s + j  ->  [t, s, j] order
    qv = np.concatenate([
        np.asarray(res[c]["q"]).reshape(128, NT, M2).transpose(1, 2, 0)
        .reshape(-1)
        for c in range(NCORES)])

    out = np.zeros(B, np.float32)
    flat_sel = sel.reshape(-1)
    flat_valid = valid.reshape(-1)
    out[flat_sel[flat_valid]] = qv[flat_valid] / SP2
    out += b3[idx]
    return out.astype(np.float32)
